# revision 28
# baseline (speedup 1.0000x reference)
"""Trainium2 Bass kernel for nn_BaselineAttention_25984552141259.

Problem: QKV [3, B=2, H=8, N=4096, d=64] fp32 ->
         out[b,h,n,:] = softmax(Q[b,h] @ K[b,h].T) @ V[b,h]

Sharding: B*H = 16 heads, embarrassingly parallel -> 2 heads per core on 8
NeuronCores. Host hands each core its Q^T/K^T pre-transposed (d=64 on
partitions): Q^T replicated on both partition halves, K^T packed two
key-blocks per 128 partitions (rows 0:64 = m-block 2i, rows 64:128 = m-block
2i+1), plus V in natural [N, d] layout.

Precision plan (validated on host, end-to-end rel err ~4e-3 vs 2e-2 gate):
  Q^T, K^T in fp16, pre-scaled by 2^-5 / 2^-4 on host (fp16 keeps 11
  mantissa bits; the 2^-9 product scale is undone by the exp's scale=512).
  fp16 stationary K^T is FWL-eligible. P and V' in bf16 (P spans
  e^-85..e^22 -> needs bf16 range; 0.4% rounding on softmax weights
  averages out in the PV reduction).

Device algorithm per head, processed in CHUNK PAIRS (two 512-query chunks
share one pass over the 16 pair-blocks = 32 key-blocks):
  S^T[2i:2i+2, c] = K^T-pair x Q^T_c           (PE, fp16, ROW-TILED: d=64 so
                                               the two K=64 matmuls of a
                                               pair-block run CONCURRENTLY
                                               on PE row-halves via
                                               tile_position (0,0)/(64,0) —
                                               ~2x S throughput. CHUNK-MAJOR
                                               PSUM tiles: both matmuls of a
                                               window write ONE tile
                                               [m0,m1]xchunk, so they share
                                               one buffer acquisition and
                                               the scheduler keeps them
                                               paired; exp of chunk a starts
                                               after window a, not after the
                                               whole quad)
  P^T = exp(512*S^T - 25)                      (per-(pair-block,chunk) groups
                                               split ~22/10 between ACT and
                                               DVE; DVE runs two custom
                                               8-stage ops: pass1 = cubic
                                               exp(v) poly squared, pass2 =
                                               8 squarings -> exp(512 v))
  O^T_{a,b}[d', n] += V'[m]^T x P^T[m, {a,b}]  (PE, V' = [V | ones] bf16;
                                               row 64 = softmax denominator.
                                               PVs are emitted with an
                                               ENGINE-AWARE LAG — ACT groups
                                               1 pair-block, DVE groups 2 —
                                               so the PE FIFO never blocks
                                               on an exp still in flight)
  copy O^T PSUM -> SBUF (o_a on ACT, o_b on DVE — both engines are
  exp-saturated so the evacuation load is split), DMA all 65 rows; host
  divides rows 0:64 by row 64 while unsharding (normalization off-device).

A few dummy matmuls + a dummy exp run during the initial DMA wait to ramp
the PE p-state clock and pull the ACT table load off the critical path.

Measured engine balance at 237.5us: PE stream ~176us (S halved by row
tiling), ACT ~195us, DVE ~191us — a balanced 3-engine pipeline; the exp
throughput (ACT 1.107us + DVE 2.44us per [128,1024] group) is the binding
constraint, and PSUM (8 banks: 6 score + 2 accumulator) caps the pipeline
depth at 3 score buffers.
"""
import numpy as np
import ml_dtypes
from contextlib import ExitStack

import concourse.bass as bass
import concourse.tile as tile
from concourse import bacc, mybir
from concourse import dve_ops as _dvo
from concourse.bass_utils import run_bass_kernel_spmd
from concourse.dve_spec import (Spec, Src0, Src1, C0, C1, C2, C3, One, lower,
                                _spill_c3_to_src1)
from concourse.dve_uop import DveOpSpec

N_CORES = 8
B, H, N, D = 2, 8, 4096, 64
HEADS = B * H
HPC = HEADS // N_CORES          # heads per core = 2
NCHUNK = 512                    # n-tile (matmul moving free dim)
NPAIR = N // (2 * NCHUNK)       # 4 chunk-pairs per head
MB = N // 128                   # 32 m-blocks of 128 keys
MBP = MB // 2                   # 16 pair-blocks (2 m-blocks row-tiled)
KQUARTER = MB // 4              # V m-blocks per load piece
PQUARTER = MBP // 4             # K^T pair-blocks per load piece
EXP_BIAS = -25.0
QSCALE = np.float32(2.0 ** -5)
KSCALE = np.float32(2.0 ** -4)  # product scale 2^-9; exp scale undoes it
# m-blocks whose exp runs on DVE (spread: clustering serializes the DVE
# queue and stalls the PE via s_ps slot starvation)
# 10 groups, spacing 3: measured optimum. 9 groups overloads ACT (+5us);
# shifting the set to {0..27} or {3..27} is also slower — the m=29 tail
# group's small spill into the next pair costs less than any alternative.
DVE_MS = frozenset({2, 5, 8, 11, 14, 17, 20, 23, 26, 29})
# last pair of the last head: put DVE groups early so the kernel tail is
# short ACT groups instead of a long DVE chain (keep spacing >= 3 m-blocks:
# the DVE service time is ~2.4us vs ~0.85us of PE work per m-block)
DVE_MS_LAST = frozenset({1, 4, 7, 10, 13, 16, 19, 22})

F32 = mybir.dt.float32
F32R = mybir.dt.float32r
F16 = mybir.dt.float16
BF16 = mybir.dt.bfloat16

_CACHE = {}

# (walrus --enable-ldw-opt is incompatible with concourse's pre-split
# InstLdweights emission — verified to fail codegen; leave it off.)


def _register_exp_ops():
    """Register the two custom DVE exp ops (in-process; the per-NEFF DVE
    table is generated from dve_ops.OPS at compile time).

    pass1: v = in0 + C1;  q = 1 + v(1 + v(C3 + v*C2));  out = q*q
           with C1 = -25/512, C2 = 1/6, C3 = 0.5 (via in1 spill)
           => out = exp(2v) * (1 + O(v^4/24))
    pass2: out = in0^256  (8 squarings)
    Chain: exp(512*in - 25), rel err < ~7e-4 over the relevant score range.
    """
    if hasattr(_dvo, "ANT_EXP_P1"):
        return _dvo.ANT_EXP_P1, _dvo.ANT_EXP_P2

    v = Src0 + C1
    q = One + v * (One + v * (C3 + v * C2))
    spec1 = Spec(
        body=_spill_c3_to_src1(q * q),
        reference=lambda in0, in1, s0, s1, imm2: (
            lambda vv: (1 + vv * (1 + vv * (np.float32(in1) + vv * np.float32(imm2)))) ** 2
        )(np.asarray(in0, np.float32) + np.float32(s1)),
    )
    x = Src0
    for _ in range(8):
        x = x * x
    spec2 = Spec(body=x)

    ops = []
    for name, spec, rd1 in (("ANT_EXP_P1", spec1, True),
                            ("ANT_EXP_P2", spec2, False)):
        row = _dvo._CUSTOM_DVE_ROW_BASE + len(_dvo.OPS)
        assert row < 0x20, "custom DVE opcode rows exhausted"
        shas = {}
        for ver in ("v3", "v4"):
            u = lower(spec, ver=ver)
            shas[ver] = DveOpSpec(name=name, opcode=row, uops=u,
                                  rd1_en=rd1).sha(ver)
        op = _dvo.DveOp(name, spec, subdim=False, uops_sha=shas)
        _dvo.OPS.append(op)
        _dvo.CUSTOM_DVE_SPECS[name] = spec
        _dvo._SUB_OPCODE_FOR_NAME[name] = row
        setattr(_dvo, name, op)
        ops.append(op)
    return ops


def _build():
    exp_p1, exp_p2 = _register_exp_ops()
    nc = bacc.Bacc("TRN2", target_bir_lowering=False, debug=False,
                   num_devices=N_CORES)
    qt_d = nc.dram_tensor("qt", [HPC, 128, N], F16, kind="ExternalInput").ap()
    kt_d = nc.dram_tensor("kt", [HPC, 128, N // 2], F16, kind="ExternalInput").ap()
    v_d = nc.dram_tensor("v", [HPC, N, D], BF16, kind="ExternalInput").ap()
    ot_d = nc.dram_tensor("ot", [HPC, D + 1, N], F32, kind="ExternalOutput").ap()

    with tile.TileContext(nc) as tc, ExitStack() as ctx:
        const = ctx.enter_context(tc.tile_pool(name="const", bufs=1))
        qk = ctx.enter_context(tc.tile_pool(name="qk", bufs=2))
        vpool = ctx.enter_context(tc.tile_pool(name="vpool", bufs=2))
        pexp = ctx.enter_context(tc.tile_pool(name="pexp", bufs=12))
        tmpp = ctx.enter_context(tc.tile_pool(name="tmpp", bufs=4))
        opool = ctx.enter_context(tc.tile_pool(name="opool", bufs=4))
        s_ps = ctx.enter_context(tc.tile_pool(name="s_ps", bufs=3, space="PSUM"))
        ot_ps = ctx.enter_context(tc.tile_pool(name="ot_ps", bufs=1, space="PSUM"))

        bias_t = const.tile([128, 1], F32)
        nc.vector.memset(bias_t[:], EXP_BIAS)
        half_t = const.tile([128, 1], F32)
        nc.vector.memset(half_t[:], 0.5)

        # PE p-state warmup + ACT table preload during the initial DMA wait:
        # ~12 x 512-row dummy matmuls keep the PE busy past the HAM ramp; a
        # tiny dummy exp pulls the ~1.3us ACT_TABLE_LOAD off the first group.
        # memset on vector: starts the warmup ~2us earlier than gpsimd
        # (whose engine init takes ~7us). An earlier 285us measurement of
        # this variant was a device-downclock artifact (ACT clock probe
        # 233us), not the code.
        warm = const.tile([128, NCHUNK], F16)
        nc.vector.memset(warm[:], 0.0)
        wtmp = const.tile([1, 8], F32)
        with nc.named_scope("warmup"):
            for w in range(6):
                ws = s_ps.tile([128, 2, NCHUNK], F32, tag="s",
                               name=f"warm{w}")
                for j in range(2):
                    nc.tensor.matmul(ws[:, j, :], warm[:, 0:128], warm[:],
                                     start=True, stop=True)
            nc.scalar.activation(wtmp[:], bias_t[0:1, :].to_broadcast((1, 8)),
                                 mybir.ActivationFunctionType.Exp,
                                 bias=bias_t[0:1, :], scale=1.0)

        kt_all, qt_all, v_all = [], [], []
        for h in range(HPC):
            with nc.named_scope(f"load{h}"):
                # split loads so the first m-blocks/chunks arrive (and
                # compute starts) before the rest of the head lands
                kt_s = []
                qt_s = []
                v_s = []
                v_re = v_d[h].rearrange("(t p) d -> p t d", p=128)
                for i in range(4):
                    kq = qk.tile([128, PQUARTER, 128], F16, tag=f"kt{i}",
                                 name=f"kt_{h}_{i}")
                    nc.sync.dma_start(
                        kq[:],
                        kt_d[h, :, bass.ts(i, PQUARTER * 128)].rearrange(
                            "p (t q) -> p t q", q=128),
                    )
                    kt_s.append(kq)
                    qq = qk.tile([128, 2, NCHUNK], F16, tag=f"qt{i}",
                                 name=f"qt_{h}_{i}")
                    nc.sync.dma_start(
                        qq[:],
                        qt_d[h, :, bass.ts(i, 2 * NCHUNK)].rearrange(
                            "p (t q) -> p t q", q=NCHUNK),
                    )
                    qt_s.append(qq)
                    # V' piece [m-part, m-tile, d+1]; col 64 = 1.0 (row sums)
                    vq = vpool.tile([128, KQUARTER, D + 1], BF16, tag=f"v{i}",
                                    name=f"v_{h}_{i}")
                    nc.sync.dma_start(
                        vq[:, :, 0:D],
                        v_re[:, bass.ts(i, KQUARTER), :],
                    )
                    nc.vector.memset(vq[:, :, D], 1.0)
                    v_s.append(vq)
                kt_all.append(kt_s)
                qt_all.append(qt_s)
                v_all.append(v_s)

        # Cross-pair software pipeline: lagged PVs carry across pair and
        # head boundaries (global pair-block counter), so a pair's tail PVs
        # drain under the NEXT pair's S quads instead of stalling the PE at
        # the boundary. Each pair's PSUM->SBUF copies are emitted the moment
        # its last PV retires.
        pend = []     # (ready_gpb, pb_src, chunk, p_t, ctx)
        # ctx = [ot_a, ot_b, v_s, h, cha, chb, groups_left]

        def pv_mm(ctx, chunk, m, p_col):
            v_c = ctx[2][m // KQUARTER][:, m % KQUARTER, :]
            nc.tensor.matmul(ctx[chunk][:], v_c, p_col,
                             start=(m == 0), stop=(m == MB - 1))

        def finish_group(ctx):
            ctx[6] -= 1
            if ctx[6] == 0:
                c_ota, c_otb, _, c_h, c_cha, c_chb, _ = ctx
                # split the two evacuation copies across ACT and DVE: both
                # engines are exp-saturated, so the load is halved per
                # engine.
                o_a = opool.tile([D + 1, NCHUNK], F32, tag="o", name="o_a")
                nc.scalar.copy(o_a[:], c_ota[:])
                nc.sync.dma_start(ot_d[c_h][:, bass.ts(c_cha, NCHUNK)],
                                  o_a[:])
                o_b = opool.tile([D + 1, NCHUNK], F32, tag="o", name="o_b")
                nc.vector.tensor_copy(o_b[:], c_otb[:])
                nc.sync.dma_start(ot_d[c_h][:, bass.ts(c_chb, NCHUNK)],
                                  o_b[:])

        def emit_group(pb_src, chunk, p_t, ctx):
            for j, m in ((0, 2 * pb_src), (1, 2 * pb_src + 1)):
                pv_mm(ctx, chunk, m, p_t[:, j, :])
            finish_group(ctx)

        def drain(gpb_now):
            # per-(pair, chunk) FIFO: a group may only drain if no earlier
            # group of the same accumulator is still pending — keeps the
            # start=True PV first and stop=True last per ot bank even when
            # mixed ACT/DVE lags would reorder readiness.
            ready, still, blocked = [], [], set()
            for ent in pend:
                key = (id(ent[4]), ent[2])
                if (key not in blocked
                        and (gpb_now is None or ent[0] <= gpb_now)):
                    ready.append(ent)
                else:
                    blocked.add(key)
                    still.append(ent)
            pend[:] = still
            # m-major across chunks when both chunks of a pair-block are
            # ready together: consecutive PVs share the same V' stationary,
            # so the duplicate LDWEIGHTS hides in the previous PV's drain.
            i = 0
            while i < len(ready):
                _, pb_src, chunk, p_t, ctx = ready[i]
                if (i + 1 < len(ready)
                        and ready[i + 1][1] == pb_src
                        and ready[i + 1][2] != chunk
                        and ready[i + 1][4] is ctx):
                    p2 = ready[i + 1][3]
                    pts = (p_t, p2) if chunk == 0 else (p2, p_t)
                    for j, m in ((0, 2 * pb_src), (1, 2 * pb_src + 1)):
                        v_c = ctx[2][m // KQUARTER][:, m % KQUARTER, :]
                        nc.tensor.matmul(
                            ctx[0][:], v_c, pts[0][:, j, :],
                            start=(m == 0), stop=(m == MB - 1))
                        nc.tensor.matmul(
                            ctx[1][:], v_c, pts[1][:, j, :],
                            start=(m == 0), stop=(m == MB - 1))
                    finish_group(ctx)
                    finish_group(ctx)
                    i += 2
                else:
                    emit_group(pb_src, chunk, p_t, ctx)
                    i += 1

        gpb = 0
        for h in range(HPC):
            kt_s, qt_s, v_s = kt_all[h], qt_all[h], v_all[h]
            with nc.named_scope(f"head{h}"):
                for pr in range(NPAIR):
                    cha, chb = 2 * pr, 2 * pr + 1
                    qa = qt_s[pr][:, 0, :]
                    qb = qt_s[pr][:, 1, :]
                    dve_ms = DVE_MS
                    if h == HPC - 1 and pr == NPAIR - 1:
                        dve_ms = DVE_MS_LAST
                    ot_a = ot_ps.tile([D + 1, NCHUNK], F32, tag="ota",
                                      name=f"ota_{h}_{pr}")
                    ot_b = ot_ps.tile([D + 1, NCHUNK], F32, tag="otb",
                                      name=f"otb_{h}_{pr}")
                    ctx = [ot_a, ot_b, v_s, h, cha, chb, 2 * MBP]

                    for pb in range(MBP):
                        # row-tiled S: K^T pair-block holds m-block 2*pb on
                        # partitions 0:64 and 2*pb+1 on 64:128; the two
                        # K=64 matmuls run concurrently on PE row-halves
                        # (tile_position auto-derived from base_partition).
                        # CHUNK-MAJOR S tiles: window a's two matmuls write
                        # the SAME tile (sa = [m0, m1] scores for chunk a),
                        # so both acquire one buffer and become ready
                        # together -> the scheduler keeps them adjacent and
                        # they run concurrently. Also lets exp(sa) start
                        # after window a instead of after the whole quad.
                        kt_c = kt_s[pb // PQUARTER][:, pb % PQUARTER, :]
                        sa = s_ps.tile([128, 2, NCHUNK], F32, tag="s",
                                       name=f"sa_{h}_{pr}_{pb}")
                        sb = s_ps.tile([128, 2, NCHUNK], F32, tag="s",
                                       name=f"sb_{h}_{pr}_{pb}")
                        wa0 = nc.tensor.matmul(sa[:, 0, :], kt_c[0:64, :],
                                               qa[0:64, :],
                                               start=True, stop=True)
                        wa1 = nc.tensor.matmul(sa[:, 1, :], kt_c[64:128, :],
                                               qa[64:128, :],
                                               start=True, stop=True)
                        wb0 = nc.tensor.matmul(sb[:, 0, :], kt_c[0:64, :],
                                               qb[0:64, :],
                                               start=True, stop=True)
                        wb1 = nc.tensor.matmul(sb[:, 1, :], kt_c[64:128, :],
                                               qb[64:128, :],
                                               start=True, stop=True)
                        for prev, nxt in ((wa0, wa1), (wa1, wb0),
                                          (wb0, wb1)):
                            tile.add_dep_helper(nxt.ins, prev.ins, sync=False,
                                                reason="pin S quad order")
                        for chunk, s_t in ((0, sa), (1, sb)):
                            slot = 2 * pb + chunk
                            p_t = pexp.tile([128, 2, NCHUNK], BF16, tag="p")
                            if slot in dve_ms:
                                t_t = tmpp.tile([128, 2, NCHUNK], F32,
                                                tag="exptmp")
                                nc.vector._custom_dve(
                                    exp_p1, out=t_t[:], in0=s_t[:],
                                    in1=half_t[:],
                                    s1=float(EXP_BIAS / 512.0),
                                    imm2=float(1.0 / 6.0),
                                )
                                nc.vector._custom_dve(
                                    exp_p2, out=p_t[:], in0=t_t[:],
                                )
                                lag = 2   # DVE chain latency ~2.4us = 2 pbs
                            else:
                                nc.scalar.activation(
                                    p_t[:], s_t[:],
                                    mybir.ActivationFunctionType.Exp,
                                    bias=bias_t[:], scale=512.0,
                                )
                                lag = 1   # ACT exp latency ~1.1us = 1 pb
                            pend.append((gpb + lag, pb, chunk, p_t, ctx))
                        # PV LAG: emit a group's PVs only after enough later
                        # S quads that its exp (ACT ~1.1us, DVE ~2.4us) has
                        # drained — the PE FIFO never blocks at a PV whose P
                        # isn't ready. Lagged PVs carry across pair/head
                        # boundaries via the global gpb counter.
                        drain(gpb)
                        gpb += 1
        drain(None)

    nc.compile()
    return nc


def _get_nc():
    if "nc" not in _CACHE:
        _CACHE["nc"] = _build()
    return _CACHE["nc"]


def _make_in_maps(QKV):
    QKV = np.asarray(QKV, dtype=np.float32)
    q = QKV[0].reshape(HEADS, N, D)
    k = QKV[1].reshape(HEADS, N, D)
    v = QKV[2].reshape(HEADS, N, D)
    # Q^T replicated on both partition halves: the two row-tiled K=64 S
    # matmuls read partitions 0:64 / 64:128 of the same moving stream.
    qt = np.empty((HEADS, 128, N), np.float16)
    qt[:, :D] = (q.transpose(0, 2, 1) * QSCALE).astype(np.float16)
    qt[:, D:] = qt[:, :D]
    # K^T packed by pair-block: partitions 0:64 = m-block 2i, 64:128 =
    # m-block 2i+1 (halves the kt DMA vs the old zero-padded layout).
    ktT = (k.transpose(0, 2, 1) * KSCALE).astype(np.float16)   # [HEADS,64,N]
    ktm = ktT.reshape(HEADS, D, MBP, 2, 128)
    kt = np.empty((HEADS, 128, N // 2), np.float16)
    kt4 = kt.reshape(HEADS, 128, MBP, 128)
    kt4[:, :D] = ktm[:, :, :, 0, :]
    kt4[:, D:] = ktm[:, :, :, 1, :]
    v16 = v.astype(ml_dtypes.bfloat16)
    in_maps = []
    for c in range(N_CORES):
        sl = slice(c * HPC, (c + 1) * HPC)
        in_maps.append({
            "qt": qt[sl],
            "kt": kt[sl],
            "v": np.ascontiguousarray(v16[sl]),
        })
    return in_maps


def _assemble(results):
    ot = np.stack([r["ot"] for r in results])            # [8, 2, 65, 4096]
    ot = ot.reshape(HEADS, D + 1, N)
    out = ot[:, 0:D, :] / ot[:, D:D + 1, :]              # normalize on host
    out = out.transpose(0, 2, 1)                         # [16, 4096, 64]
    return np.ascontiguousarray(out).reshape(B, H, N, D).astype(np.float32)


def kernel(QKV):
    nc = _get_nc()
    res = run_bass_kernel_spmd(nc, _make_in_maps(QKV), list(range(N_CORES)))
    return _assemble(res.results)



# revision 31
# speedup vs baseline: 1.2134x; 1.2134x over previous
"""Trainium2 Bass kernel for nn_BaselineAttention_25984552141259.

Problem: QKV [3, B=2, H=8, N=4096, d=64] fp32 ->
         out[b,h,n,:] = softmax(Q[b,h] @ K[b,h].T) @ V[b,h]

Sharding: B*H = 16 heads, embarrassingly parallel -> 2 heads per core on 8
NeuronCores. Host hands each core its Q^T/K^T pre-transposed (d=64 on
partitions): Q^T replicated on both partition halves, K^T packed two
key-blocks per 128 partitions (rows 0:64 = m-block 2i, rows 64:128 = m-block
2i+1), plus V in natural [N, d] layout.

Precision plan (validated on host, end-to-end rel err ~4e-3 vs 2e-2 gate):
  Q^T, K^T in fp16, pre-scaled by 2^-5 / 2^-4 on host (fp16 keeps 11
  mantissa bits; the 2^-9 product scale is undone by the exp's scale=512).
  fp16 stationary K^T is FWL-eligible. P and V' in bf16 (P spans
  e^-85..e^22 -> needs bf16 range; 0.4% rounding on softmax weights
  averages out in the PV reduction).

Device algorithm per head, processed in CHUNK PAIRS (two 512-query chunks
share one pass over the 16 pair-blocks = 32 key-blocks):
  S^T[2i:2i+2, c] = K^T-pair x Q^T_c           (PE, fp16, ROW-TILED: d=64 so
                                               the two K=64 matmuls of a
                                               pair-block run CONCURRENTLY
                                               on PE row-halves via
                                               tile_position (0,0)/(64,0) —
                                               ~2x S throughput. CHUNK-MAJOR
                                               PSUM tiles: both matmuls of a
                                               window write ONE tile
                                               [m0,m1]xchunk, so they share
                                               one buffer acquisition and
                                               the scheduler keeps them
                                               paired; exp of chunk a starts
                                               after window a, not after the
                                               whole quad)
  P^T = exp(512*S^T - 25)                      (per-(pair-block,chunk) groups
                                               split ~22/10 between ACT and
                                               DVE; DVE runs two custom
                                               8-stage ops: pass1 = cubic
                                               exp(v) poly squared, pass2 =
                                               8 squarings -> exp(512 v))
  O^T_{a,b}[d', n] += V'[m]^T x P^T[m, {a,b}]  (PE, V' = [V | ones] bf16;
                                               row 64 = softmax denominator.
                                               PVs are emitted with an
                                               ENGINE-AWARE LAG — ACT groups
                                               1 pair-block, DVE groups 2 —
                                               so the PE FIFO never blocks
                                               on an exp still in flight)
  copy O^T PSUM -> SBUF (o_a on ACT, o_b on DVE — both engines are
  exp-saturated so the evacuation load is split), DMA all 65 rows; host
  divides rows 0:64 by row 64 while unsharding (normalization off-device).

A few dummy matmuls + a dummy exp run during the initial DMA wait to ramp
the PE p-state clock and pull the ACT table load off the critical path.

Measured engine balance at 237.5us: PE stream ~176us (S halved by row
tiling), ACT ~195us, DVE ~191us — a balanced 3-engine pipeline; the exp
throughput (ACT 1.107us + DVE 2.44us per [128,1024] group) is the binding
constraint, and PSUM (8 banks: 6 score + 2 accumulator) caps the pipeline
depth at 3 score buffers.
"""
import numpy as np
import ml_dtypes
from contextlib import ExitStack

import concourse.bass as bass
import concourse.tile as tile
from concourse import bacc, mybir
from concourse import dve_ops as _dvo
from concourse.bass_utils import run_bass_kernel_spmd
from concourse.dve_spec import (Spec, Src0, Src1, C0, C1, C2, C3, One, lower,
                                _spill_c3_to_src1)
from concourse.dve_uop import DveOpSpec

N_CORES = 8
B, H, N, D = 2, 8, 4096, 64
HEADS = B * H
HPC = HEADS // N_CORES          # heads per core = 2
NCHUNK = 512                    # n-tile (matmul moving free dim)
NPAIR = N // (2 * NCHUNK)       # 4 chunk-pairs per head
MB = N // 128                   # 32 m-blocks of 128 keys
MBP = MB // 2                   # 16 pair-blocks (2 m-blocks row-tiled)
KQUARTER = MB // 4              # V m-blocks per load piece
PQUARTER = MBP // 4             # K^T pair-blocks per load piece
EXP_BIAS = -25.0
QSCALE = np.float32(2.0 ** -5)
KSCALE = np.float32(2.0 ** -4)  # product scale 2^-9; exp scale undoes it
# m-blocks whose exp runs on DVE (spread: clustering serializes the DVE
# queue and stalls the PE via s_ps slot starvation)
# 10 groups, spacing 3: measured optimum. 9 groups overloads ACT (+5us);
# shifting the set to {0..27} or {3..27} is also slower — the m=29 tail
# group's small spill into the next pair costs less than any alternative.
DVE_MS = frozenset({2, 5, 8, 11, 14, 17, 20, 23, 26, 29})
# last pair of the last head: put DVE groups early so the kernel tail is
# short ACT groups instead of a long DVE chain (keep spacing >= 3 m-blocks:
# the DVE service time is ~2.4us vs ~0.85us of PE work per m-block)
DVE_MS_LAST = frozenset({1, 4, 7, 10, 13, 16, 19, 22})

F32 = mybir.dt.float32
F32R = mybir.dt.float32r
F16 = mybir.dt.float16
BF16 = mybir.dt.bfloat16

_CACHE = {}

# (walrus --enable-ldw-opt is incompatible with concourse's pre-split
# InstLdweights emission — verified to fail codegen; leave it off.)


def _register_exp_ops():
    """Register the two custom DVE exp ops (in-process; the per-NEFF DVE
    table is generated from dve_ops.OPS at compile time).

    pass1: v = in0 + C1;  q = 1 + v(1 + v(C3 + v*C2));  out = q*q
           with C1 = -25/512, C2 = 1/6, C3 = 0.5 (via in1 spill)
           => out = exp(2v) * (1 + O(v^4/24))
    pass2: out = in0^256  (8 squarings)
    Chain: exp(512*in - 25), rel err < ~7e-4 over the relevant score range.
    """
    if hasattr(_dvo, "ANT_EXP_P1"):
        return _dvo.ANT_EXP_P1, _dvo.ANT_EXP_P2

    v = Src0 + C1
    q = One + v * (One + v * (C3 + v * C2))
    spec1 = Spec(
        body=_spill_c3_to_src1(q * q),
        reference=lambda in0, in1, s0, s1, imm2: (
            lambda vv: (1 + vv * (1 + vv * (np.float32(in1) + vv * np.float32(imm2)))) ** 2
        )(np.asarray(in0, np.float32) + np.float32(s1)),
    )
    x = Src0
    for _ in range(8):
        x = x * x
    spec2 = Spec(body=x)

    ops = []
    for name, spec, rd1 in (("ANT_EXP_P1", spec1, True),
                            ("ANT_EXP_P2", spec2, False)):
        row = _dvo._CUSTOM_DVE_ROW_BASE + len(_dvo.OPS)
        assert row < 0x20, "custom DVE opcode rows exhausted"
        shas = {}
        for ver in ("v3", "v4"):
            u = lower(spec, ver=ver)
            shas[ver] = DveOpSpec(name=name, opcode=row, uops=u,
                                  rd1_en=rd1).sha(ver)
        op = _dvo.DveOp(name, spec, subdim=False, uops_sha=shas)
        _dvo.OPS.append(op)
        _dvo.CUSTOM_DVE_SPECS[name] = spec
        _dvo._SUB_OPCODE_FOR_NAME[name] = row
        setattr(_dvo, name, op)
        ops.append(op)
    return ops


def _build():
    exp_p1, exp_p2 = _register_exp_ops()
    nc = bacc.Bacc("TRN2", target_bir_lowering=False, debug=False,
                   num_devices=N_CORES)
    qt_d = nc.dram_tensor("qt", [HPC, 128, N], F16, kind="ExternalInput").ap()
    kt_d = nc.dram_tensor("kt", [HPC, 128, N // 2], F16, kind="ExternalInput").ap()
    v_d = nc.dram_tensor("v", [HPC, N, D], BF16, kind="ExternalInput").ap()
    ot_d = nc.dram_tensor("ot", [HPC, D + 1, N], F32, kind="ExternalOutput").ap()

    with tile.TileContext(nc) as tc, ExitStack() as ctx:
        const = ctx.enter_context(tc.tile_pool(name="const", bufs=1))
        qk = ctx.enter_context(tc.tile_pool(name="qk", bufs=2))
        vpool = ctx.enter_context(tc.tile_pool(name="vpool", bufs=2))
        pexp = ctx.enter_context(tc.tile_pool(name="pexp", bufs=12))
        tmpp = ctx.enter_context(tc.tile_pool(name="tmpp", bufs=4))
        opool = ctx.enter_context(tc.tile_pool(name="opool", bufs=4))
        s_ps = ctx.enter_context(tc.tile_pool(name="s_ps", bufs=3, space="PSUM"))
        ot_ps = ctx.enter_context(tc.tile_pool(name="ot_ps", bufs=1, space="PSUM"))

        bias_t = const.tile([128, 1], F32)
        nc.vector.memset(bias_t[:], EXP_BIAS)
        half_t = const.tile([128, 1], F32)
        nc.vector.memset(half_t[:], 0.5)

        # PE p-state warmup + ACT table preload during the initial DMA wait:
        # ~12 x 512-row dummy matmuls keep the PE busy past the HAM ramp; a
        # tiny dummy exp pulls the ~1.3us ACT_TABLE_LOAD off the first group.
        # memset on gpsimd so the warmup doesn't queue behind DVE work
        # (nc.vector variant measured neutral after normalizing for the
        # device-downclock artifact; keep the verified config)
        warm = const.tile([128, NCHUNK], F16)
        nc.gpsimd.memset(warm[:], 0.0)
        wtmp = const.tile([1, 8], F32)
        with nc.named_scope("warmup"):
            for w in range(6):
                ws = s_ps.tile([128, 2, NCHUNK], F32, tag="s",
                               name=f"warm{w}")
                for j in range(2):
                    nc.tensor.matmul(ws[:, j, :], warm[:, 0:128], warm[:],
                                     start=True, stop=True)
            nc.scalar.activation(wtmp[:], bias_t[0:1, :].to_broadcast((1, 8)),
                                 mybir.ActivationFunctionType.Exp,
                                 bias=bias_t[0:1, :], scale=1.0)

        kt_all, qt_all, v_all = [], [], []
        for h in range(HPC):
            with nc.named_scope(f"load{h}"):
                # split loads so the first m-blocks/chunks arrive (and
                # compute starts) before the rest of the head lands
                kt_s = []
                qt_s = []
                v_s = []
                v_re = v_d[h].rearrange("(t p) d -> p t d", p=128)
                for i in range(4):
                    kq = qk.tile([128, PQUARTER, 128], F16, tag=f"kt{i}",
                                 name=f"kt_{h}_{i}")
                    nc.sync.dma_start(
                        kq[:],
                        kt_d[h, :, bass.ts(i, PQUARTER * 128)].rearrange(
                            "p (t q) -> p t q", q=128),
                    )
                    kt_s.append(kq)
                    qq = qk.tile([128, 2, NCHUNK], F16, tag=f"qt{i}",
                                 name=f"qt_{h}_{i}")
                    nc.sync.dma_start(
                        qq[:],
                        qt_d[h, :, bass.ts(i, 2 * NCHUNK)].rearrange(
                            "p (t q) -> p t q", q=NCHUNK),
                    )
                    qt_s.append(qq)
                    # V' piece [m-part, m-tile, d+1]; col 64 = 1.0 (row sums)
                    vq = vpool.tile([128, KQUARTER, D + 1], BF16, tag=f"v{i}",
                                    name=f"v_{h}_{i}")
                    nc.sync.dma_start(
                        vq[:, :, 0:D],
                        v_re[:, bass.ts(i, KQUARTER), :],
                    )
                    nc.vector.memset(vq[:, :, D], 1.0)
                    v_s.append(vq)
                kt_all.append(kt_s)
                qt_all.append(qt_s)
                v_all.append(v_s)

        # Cross-pair software pipeline: lagged PVs carry across pair and
        # head boundaries (global pair-block counter), so a pair's tail PVs
        # drain under the NEXT pair's S quads instead of stalling the PE at
        # the boundary. Each pair's PSUM->SBUF copies are emitted the moment
        # its last PV retires.
        pend = []     # (ready_gpb, pb_src, chunk, p_t, ctx)
        # ctx = [ot_a, ot_b, v_s, h, cha, chb, groups_left]

        def pv_mm(ctx, chunk, m, p_col):
            v_c = ctx[2][m // KQUARTER][:, m % KQUARTER, :]
            nc.tensor.matmul(ctx[chunk][:], v_c, p_col,
                             start=(m == 0), stop=(m == MB - 1))

        def finish_group(ctx):
            ctx[6] -= 1
            if ctx[6] == 0:
                c_ota, c_otb, _, c_h, c_cha, c_chb, _ = ctx
                # split the two evacuation copies across ACT and DVE: both
                # engines are exp-saturated, so the load is halved per
                # engine.
                o_a = opool.tile([D + 1, NCHUNK], F32, tag="o", name="o_a")
                nc.scalar.copy(o_a[:], c_ota[:])
                nc.sync.dma_start(ot_d[c_h][:, bass.ts(c_cha, NCHUNK)],
                                  o_a[:])
                o_b = opool.tile([D + 1, NCHUNK], F32, tag="o", name="o_b")
                nc.vector.tensor_copy(o_b[:], c_otb[:])
                nc.sync.dma_start(ot_d[c_h][:, bass.ts(c_chb, NCHUNK)],
                                  o_b[:])

        def emit_group(pb_src, chunk, p_t, ctx):
            for j, m in ((0, 2 * pb_src), (1, 2 * pb_src + 1)):
                pv_mm(ctx, chunk, m, p_t[:, j, :])
            finish_group(ctx)

        def drain(gpb_now):
            # PSUM has_written only constrains the START (clears the bank)
            # and STOP (last write before the copy) PVs per accumulator —
            # middles accumulate commutatively. Enforce exactly that: no
            # group drains before its chunk's start group, and the stop
            # group drains only when nothing of its chunk is pending.
            # Middles reorder freely, letting ACT groups overtake
            # DVE-delayed ones instead of convoying behind them.
            ready, still = [], []
            pending_keys = {}
            start_blocked = set()
            for ent in pend:
                key = (id(ent[4]), ent[2])
                is_ready = gpb_now is None or ent[0] <= gpb_now
                if (is_ready and key not in start_blocked
                        and not (ent[1] == MBP - 1
                                 and pending_keys.get(key, 0) > 0)):
                    ready.append(ent)
                else:
                    still.append(ent)
                    pending_keys[key] = pending_keys.get(key, 0) + 1
                    if ent[1] == 0:
                        start_blocked.add(key)
            pend[:] = still
            # m-major across chunks when both chunks of a pair-block are
            # ready together: consecutive PVs share the same V' stationary,
            # so the duplicate LDWEIGHTS hides in the previous PV's drain.
            i = 0
            while i < len(ready):
                _, pb_src, chunk, p_t, ctx = ready[i]
                if (i + 1 < len(ready)
                        and ready[i + 1][1] == pb_src
                        and ready[i + 1][2] != chunk
                        and ready[i + 1][4] is ctx):
                    p2 = ready[i + 1][3]
                    pts = (p_t, p2) if chunk == 0 else (p2, p_t)
                    for j, m in ((0, 2 * pb_src), (1, 2 * pb_src + 1)):
                        v_c = ctx[2][m // KQUARTER][:, m % KQUARTER, :]
                        nc.tensor.matmul(
                            ctx[0][:], v_c, pts[0][:, j, :],
                            start=(m == 0), stop=(m == MB - 1))
                        nc.tensor.matmul(
                            ctx[1][:], v_c, pts[1][:, j, :],
                            start=(m == 0), stop=(m == MB - 1))
                    finish_group(ctx)
                    finish_group(ctx)
                    i += 2
                else:
                    emit_group(pb_src, chunk, p_t, ctx)
                    i += 1

        gpb = 0
        for h in range(HPC):
            kt_s, qt_s, v_s = kt_all[h], qt_all[h], v_all[h]
            with nc.named_scope(f"head{h}"):
                for pr in range(NPAIR):
                    cha, chb = 2 * pr, 2 * pr + 1
                    qa = qt_s[pr][:, 0, :]
                    qb = qt_s[pr][:, 1, :]
                    dve_ms = DVE_MS
                    if h == HPC - 1 and pr == NPAIR - 1:
                        dve_ms = DVE_MS_LAST
                    ot_a = ot_ps.tile([D + 1, NCHUNK], F32, tag="ota",
                                      name=f"ota_{h}_{pr}")
                    ot_b = ot_ps.tile([D + 1, NCHUNK], F32, tag="otb",
                                      name=f"otb_{h}_{pr}")
                    ctx = [ot_a, ot_b, v_s, h, cha, chb, 2 * MBP]

                    for pb in range(MBP):
                        # row-tiled S: K^T pair-block holds m-block 2*pb on
                        # partitions 0:64 and 2*pb+1 on 64:128; the two
                        # K=64 matmuls run concurrently on PE row-halves
                        # (tile_position auto-derived from base_partition).
                        # CHUNK-MAJOR S tiles: window a's two matmuls write
                        # the SAME tile (sa = [m0, m1] scores for chunk a),
                        # so both acquire one buffer and become ready
                        # together -> the scheduler keeps them adjacent and
                        # they run concurrently. Also lets exp(sa) start
                        # after window a instead of after the whole quad.
                        kt_c = kt_s[pb // PQUARTER][:, pb % PQUARTER, :]
                        sa = s_ps.tile([128, 2, NCHUNK], F32, tag="s",
                                       name=f"sa_{h}_{pr}_{pb}")
                        sb = s_ps.tile([128, 2, NCHUNK], F32, tag="s",
                                       name=f"sb_{h}_{pr}_{pb}")
                        wa0 = nc.tensor.matmul(sa[:, 0, :], kt_c[0:64, :],
                                               qa[0:64, :],
                                               start=True, stop=True)
                        wa1 = nc.tensor.matmul(sa[:, 1, :], kt_c[64:128, :],
                                               qa[64:128, :],
                                               start=True, stop=True)
                        wb0 = nc.tensor.matmul(sb[:, 0, :], kt_c[0:64, :],
                                               qb[0:64, :],
                                               start=True, stop=True)
                        wb1 = nc.tensor.matmul(sb[:, 1, :], kt_c[64:128, :],
                                               qb[64:128, :],
                                               start=True, stop=True)
                        for prev, nxt in ((wa0, wa1), (wa1, wb0),
                                          (wb0, wb1)):
                            tile.add_dep_helper(nxt.ins, prev.ins, sync=False,
                                                reason="pin S quad order")
                        for chunk, s_t in ((0, sa), (1, sb)):
                            slot = 2 * pb + chunk
                            p_t = pexp.tile([128, 2, NCHUNK], BF16, tag="p")
                            if slot in dve_ms:
                                t_t = tmpp.tile([128, 2, NCHUNK], F32,
                                                tag="exptmp")
                                nc.vector._custom_dve(
                                    exp_p1, out=t_t[:], in0=s_t[:],
                                    in1=half_t[:],
                                    s1=float(EXP_BIAS / 512.0),
                                    imm2=float(1.0 / 6.0),
                                )
                                nc.vector._custom_dve(
                                    exp_p2, out=p_t[:], in0=t_t[:],
                                )
                                lag = 3   # DVE chain ~2.4us + queue jitter;
                                          # middles may legally drain late
                                          # (loose guard above)
                            else:
                                nc.scalar.activation(
                                    p_t[:], s_t[:],
                                    mybir.ActivationFunctionType.Exp,
                                    bias=bias_t[:], scale=512.0,
                                )
                                lag = 1   # ACT exp latency ~1.1us = 1 pb
                            pend.append((gpb + lag, pb, chunk, p_t, ctx))
                        # PV LAG: emit a group's PVs only after enough later
                        # S quads that its exp (ACT ~1.1us, DVE ~2.4us) has
                        # drained — the PE FIFO never blocks at a PV whose P
                        # isn't ready. Lagged PVs carry across pair/head
                        # boundaries via the global gpb counter.
                        drain(gpb)
                        gpb += 1
        drain(None)

    nc.compile()
    return nc


def _get_nc():
    if "nc" not in _CACHE:
        _CACHE["nc"] = _build()
    return _CACHE["nc"]


def _make_in_maps(QKV):
    QKV = np.asarray(QKV, dtype=np.float32)
    q = QKV[0].reshape(HEADS, N, D)
    k = QKV[1].reshape(HEADS, N, D)
    v = QKV[2].reshape(HEADS, N, D)
    # Q^T replicated on both partition halves: the two row-tiled K=64 S
    # matmuls read partitions 0:64 / 64:128 of the same moving stream.
    qt = np.empty((HEADS, 128, N), np.float16)
    qt[:, :D] = (q.transpose(0, 2, 1) * QSCALE).astype(np.float16)
    qt[:, D:] = qt[:, :D]
    # K^T packed by pair-block: partitions 0:64 = m-block 2i, 64:128 =
    # m-block 2i+1 (halves the kt DMA vs the old zero-padded layout).
    ktT = (k.transpose(0, 2, 1) * KSCALE).astype(np.float16)   # [HEADS,64,N]
    ktm = ktT.reshape(HEADS, D, MBP, 2, 128)
    kt = np.empty((HEADS, 128, N // 2), np.float16)
    kt4 = kt.reshape(HEADS, 128, MBP, 128)
    kt4[:, :D] = ktm[:, :, :, 0, :]
    kt4[:, D:] = ktm[:, :, :, 1, :]
    v16 = v.astype(ml_dtypes.bfloat16)
    in_maps = []
    for c in range(N_CORES):
        sl = slice(c * HPC, (c + 1) * HPC)
        in_maps.append({
            "qt": qt[sl],
            "kt": kt[sl],
            "v": np.ascontiguousarray(v16[sl]),
        })
    return in_maps


def _assemble(results):
    ot = np.stack([r["ot"] for r in results])            # [8, 2, 65, 4096]
    ot = ot.reshape(HEADS, D + 1, N)
    out = ot[:, 0:D, :] / ot[:, D:D + 1, :]              # normalize on host
    out = out.transpose(0, 2, 1)                         # [16, 4096, 64]
    return np.ascontiguousarray(out).reshape(B, H, N, D).astype(np.float32)


def kernel(QKV):
    nc = _get_nc()
    res = run_bass_kernel_spmd(nc, _make_in_maps(QKV), list(range(N_CORES)))
    return _assemble(res.results)

